# revision 12
# baseline (speedup 1.0000x reference)
"""Trainium2 Bass kernel for CausalWanSelfAttention (KV-cache-bias attention).

Math: the reference's disjoint-segment attention + LSE merge is exactly
global softmax with a per-key bias b_l (log 0.1 on keys in
[frame_seqlen, current_block_start)).  exp needs no max-subtraction
(scores ~ N(0,1), max ~ 6), so out = (E @ V) / (1^T E) with
E = exp(scale*S + b_l).

Sharding: 24 units = (head h in 0..11, q-half in {0,1}), 3 units per core.
Each unit: 1024 queries x 1 head x all 8192 keys, 64 key chunks of 128.

Device pipeline per unit (matmuls fp16, accumulate fp32 PSUM):
  A:    S^T[l 128, q 1024] = kt-chunk^T @ qt        (N=512 MMs)
  exp:  E = exp(S^T * scale + bias) fp16            (fused ACT instrs)
  B:    O^T[d 128, q 1024] += v-chunk^T @ E         (N=512 MMs)
  R:    in-place fp16 DVE running sum of E slices -> [128,1024] per unit
The kernel is ACT(exp)-bound: Lq*Lk*H/8 = 24.6M exp elements per core at
1 elem/cycle/lane @1.2GHz is ~164us.  Everything else is arranged so ACT
never stalls: exp instructions are fused over 3 half-chunk (512-col)
blocks ([128,1536], 43 instrs/unit instead of 64, amortizing the ~350
cycle ACT instruction overhead), B lags A by a step so the in-order PE
queue always runs the A feeding the next exp first, and the norm never
touches PE/PSUM (R tile is DMA'd out; the host does the final 128-row
cross-partition sum + divide + transpose).

Fusing exp across chunks requires the bias to be constant per ACT
instruction: segment boundaries that are multiples of 128 make the bias
constant per key chunk, so it folds into a [128,1] constant tile and
instructions may span chunks within a segment.  Non-aligned boundaries
fall back to per-chunk (2-block) instructions with a per-partition bias
column DMA'd from the host.  PSUM: s tiles (2x3 banks) + O halves (2).
"""

import math
import sys

for _p in ("/opt/trn_rl_repo",):
    if _p not in sys.path:
        sys.path.insert(0, _p)

import numpy as np

import concourse.bass as bass
import concourse.mybir as mybir
import concourse.tile as tile
from concourse import bacc
from concourse.bass_utils import run_bass_kernel_spmd

F16 = mybir.dt.float16
F32 = mybir.dt.float32

B, LQ, LK, H, D = 1, 2048, 8192, 12, 128
N_CORES = 8
UNITS_PER_CORE = 3          # 24 units = 12 heads x 2 q-halves
QSPAN = 1024                # queries per unit
NLC = LK // 128             # 64 key chunks of 128
SCALE = 1.0 / math.sqrt(D)
LOG_BIAS = math.log(0.1)

_CACHED = None
TIME_LOOP = 1     # timing experiments only: hardware-loop the body N times


def _plan_tiles(fe, bs):
    """Per-unit exp-tile plan: list of (bias, [(chunk, half), ...]).

    Aligned fe/bs: tiles of up to 3 half-chunk blocks, never spanning a
    bias-segment boundary (bias is a compile-time constant per tile).
    Non-aligned: one 2-block tile per chunk with bias=None (use the
    per-partition bias column for that chunk)."""
    if fe % 128 == 0 and bs % 128 == 0:
        tiles = []
        for lo, hi, b in ((0, fe // 128, 0.0),
                          (fe // 128, bs // 128, LOG_BIAS),
                          (bs // 128, NLC, 0.0)):
            blocks = [(c, h) for c in range(lo, hi) for h in range(2)]
            for i in range(0, len(blocks), 3):
                tiles.append((b, blocks[i:i + 3]))
        return tiles, True
    return [(None, [(c, 0), (c, 1)]) for c in range(NLC)], False


def _radds(blocks):
    """Greedy aligned add list for accumulating a tile's E blocks into
    R[128,1024]: returns [(dst_off, src_off, width)]."""
    adds, i = [], 0
    while i < len(blocks):
        c, h = blocks[i]
        if h == 0 and i + 1 < len(blocks) and blocks[i + 1] == (c, 1):
            adds.append((0, i * 512, 1024))
            i += 2
        else:
            adds.append((h * 512, i * 512, 512))
            i += 1
    return adds


def _build_program(fe, bs):
    nc = bacc.Bacc("TRN2", target_bir_lowering=False, debug=False,
                   enable_asserts=False)

    qt_d = nc.dram_tensor("qt", [UNITS_PER_CORE, 128, QSPAN], F16,
                          kind="ExternalInput")
    kt_d = nc.dram_tensor("kt", [UNITS_PER_CORE, 128, LK], F16,
                          kind="ExternalInput")
    # vl host layout: [u, p, c*d] — per-partition contiguous so DMA runs
    # long descriptor lines (p = key index within chunk, c = chunk)
    vl_d = nc.dram_tensor("vl", [UNITS_PER_CORE, 128, NLC * 128], F16,
                          kind="ExternalInput")
    bias_d = nc.dram_tensor("bias", [128, NLC], F32, kind="ExternalInput")
    ot_d = nc.dram_tensor("ot", [UNITS_PER_CORE, 128, QSPAN], F16,
                          kind="ExternalOutput")
    tr_d = nc.dram_tensor("tr", [UNITS_PER_CORE, 128, QSPAN], F16,
                          kind="ExternalOutput")

    qt_ap = qt_d.ap()
    kt_ap = kt_d.ap()
    vl_ap = vl_d.ap().rearrange("u p (c d) -> u p c d", d=128)
    bias_ap = bias_d.ap()
    ot_ap = ot_d.ap()
    tr_ap = tr_d.ap()

    utiles, aligned = _plan_tiles(fe, bs)
    NT = len(utiles)
    swidth = max(len(b) for _, b in utiles) * 512
    # chunk -> {half: (tile, col_off)}
    half_src = {}
    for t, (_b, blocks) in enumerate(utiles):
        for i, (c, h) in enumerate(blocks):
            half_src.setdefault(c, {})[h] = (t, i * 512)

    with tile.TileContext(nc) as tc:
        with (
            tc.tile_pool(name="kt_pool", bufs=2) as kt_pool,
            tc.tile_pool(name="vl_pool", bufs=2) as vl_pool,
            tc.tile_pool(name="qt_pool", bufs=2) as qt_pool,
            tc.tile_pool(name="cn_pool", bufs=1) as cn_pool,
            tc.tile_pool(name="e_pool", bufs=4) as e_pool,
            tc.tile_pool(name="r_pool", bufs=2) as r_pool,
            tc.tile_pool(name="ob_pool", bufs=2) as ob_pool,
            tc.tile_pool(name="s_pool", bufs=2, space="PSUM") as s_pool,
            tc.tile_pool(name="o_pool", bufs=1, space="PSUM") as o_pool,
        ):
            # Warm the ACT exp table before any DMA-dependent work so the
            # ~1.3us table load overlaps the first unit's input DMA.
            warm_t = cn_pool.tile([128, 1], F32, name="warm_t")
            nc.vector.memset(warm_t[:], 0.0)
            nc.scalar.activation(warm_t[:], warm_t[:],
                                 mybir.ActivationFunctionType.Exp)
            bias_c = {}
            if aligned:
                for bv in set(b for b, _ in utiles):
                    bt = cn_pool.tile([128, 1], F32,
                                      name=f"bias_{len(bias_c)}")
                    nc.vector.memset(bt[:], bv)
                    bias_c[bv] = bt
            else:
                bias_t = cn_pool.tile([128, NLC], F32, name="bias_t")

            import contextlib
            loop_cm = (tc.For_i(0, TIME_LOOP, 1) if TIME_LOOP > 1
                       else contextlib.nullcontext())

            loaded = {}

            def load_unit(u):
                # qt first (every tile needs it), then k/v interleaved in
                # eighths so tile 0's compute starts after ~1/8 of the
                # load.  For unit 0 the first chunks ride ahead as a tiny
                # DMA so A(tile 0) isn't gated on a full eighth.
                qt = qt_pool.tile([128, QSPAN], F16, name=f"qt_u{u}", tag="qt")
                nc.sync.dma_start(out=qt[:], in_=qt_ap[u])
                kt = kt_pool.tile([128, LK], F16, name=f"kt_u{u}", tag="kt")
                vl = vl_pool.tile([128, NLC, 128], F16,
                                  name=f"vl_u{u}", tag="vl")
                first = 0
                if u == 0:
                    nc.sync.dma_start(out=kt[:, 0:256], in_=kt_ap[u][:, 0:256])
                    if not aligned:
                        nc.sync.dma_start(out=bias_t[:], in_=bias_ap)
                    first = 256
                for eighth in range(8):
                    slk = slice(max(first, eighth * (LK // 8)),
                                (eighth + 1) * (LK // 8))
                    nc.sync.dma_start(out=kt[:, slk], in_=kt_ap[u][:, slk])
                    slv = bass.ts(eighth, NLC // 8)
                    nc.sync.dma_start(out=vl[:, slv, :], in_=vl_ap[u][:, slv, :])
                loaded[u] = (kt, vl, qt)

            NGT = UNITS_PER_CORE * NT

            with loop_cm:
                load_unit(0)
                # Global software-pipelined stream over exp tiles.  Per
                # step t: A-blocks of tile t, exp(t), then B for chunks
                # fully available in tiles <= t-1 (B lags so PE's in-order
                # queue never delays the A feeding the next exp).
                cur, ot_t, rt = {}, {}, {}
                etiles = {}
                next_b = 0

                def emit_b(ud, dl):
                    kt, vl, qt = cur[ud]
                    if dl == 0:
                        ot_t[ud] = [o_pool.tile([128, 512], F32,
                                                name=f"ot_u{ud}h{h}",
                                                tag=f"ot{h}")
                                    for h in range(2)]
                    if dl == NLC - 1:
                        nc.sync.dma_start(out=tr_ap[ud], in_=rt[ud][:])
                    for h in range(2):
                        et, off = half_src[dl][h]
                        nc.tensor.matmul(
                            ot_t[ud][h][:],
                            lhsT=vl[:, dl, :],
                            rhs=etiles[(ud, et)][:, off:off + 512],
                            start=(dl == 0), stop=(dl == NLC - 1))
                        if dl == NLC - 1:
                            # evacuate/DMA each half as soon as its group
                            # closes so the tail overlaps the other half
                            ot_sb = ob_pool.tile([128, 512], F16,
                                                 name=f"otsb_u{ud}h{h}",
                                                 tag="otsb")
                            nc.vector.tensor_scalar_add(
                                ot_sb[:], ot_t[ud][h][:], 0.0)
                            nc.sync.dma_start(
                                out=ot_ap[ud][:, bass.ts(h, 512)],
                                in_=ot_sb[:])
                    if dl == NLC - 1:
                        ot_t.pop(ud)

                for g in range(NGT + 2):
                    if g < NGT:
                        ug, tg = g // NT, g % NT
                        if tg == 0:
                            cur[ug] = loaded.pop(ug)
                            rt[ug] = r_pool.tile([128, QSPAN], F16,
                                                 name=f"r_u{ug}", tag="r")
                            nc.vector.memset(rt[ug][:], 0.0)
                        kt, vl, qt = cur[ug]
                        b, blocks = utiles[tg]
                        width = len(blocks) * 512
                        s = s_pool.tile([128, swidth], F32)
                        for i, (c, h) in enumerate(blocks):
                            nc.tensor.matmul(
                                s[:, i * 512:(i + 1) * 512],
                                lhsT=kt[:, bass.ts(c, 128)],
                                rhs=qt[:, bass.ts(h, 512)],
                                start=True, stop=True)
                        e = e_pool.tile([128, swidth], F16)
                        bias_arg = (bias_c[b][:] if aligned
                                    else bias_t[:, blocks[0][0]:blocks[0][0] + 1])
                        nc.scalar.activation(
                            e[:, :width], s[:, :width],
                            mybir.ActivationFunctionType.Exp,
                            bias=bias_arg, scale=SCALE)
                        etiles[(ug, tg)] = e
                        r = rt[ug]
                        for dst, src, w in _radds(blocks):
                            nc.vector.tensor_add(
                                r[:, dst:dst + w], r[:, dst:dst + w],
                                e[:, src:src + w])
                        if tg == 6 and ug + 1 < UNITS_PER_CORE:
                            load_unit(ug + 1)  # prefetch next unit
                    # B emission: chunks whose blocks live in tiles
                    # emitted at least one step ago
                    t_done = g - 1
                    while next_b < UNITS_PER_CORE * NLC:
                        ud, dl = next_b // NLC, next_b % NLC
                        lim = max(half_src[dl][0][0], half_src[dl][1][0])
                        if ud * NT + lim <= t_done - 1:
                            emit_b(ud, dl)
                            next_b += 1
                        else:
                            break
                    # free e tiles fully consumed by B
                    done_c = next_b - 1
                    if done_c >= 0:
                        cut = ((done_c // NLC) * NT +
                               min(half_src[done_c % NLC][0][0],
                                   half_src[done_c % NLC][1][0]))
                        for key in [k for k in etiles
                                    if k[0] * NT + k[1] < cut]:
                            etiles.pop(key)

    nc.compile()
    return nc


def _get_program(fe=1536, bs=6144):
    global _CACHED
    if _CACHED is None:
        _CACHED = {}
    key = (fe, bs, TIME_LOOP)
    if key not in _CACHED:
        _CACHED[key] = _build_program(fe, bs)
    return _CACHED[key]


def _host_prep(q, k, v, frame_seqlen, current_block_start):
    fs = max(0, min(int(frame_seqlen), LK))
    bs = max(0, min(int(current_block_start), LK))
    logw = np.zeros(LK, np.float32)
    logw[fs:bs] = LOG_BIAS
    bias = np.ascontiguousarray(logw.reshape(NLC, 128).T)  # [128, NLC]

    q = np.asarray(q, dtype=np.float32)
    k = np.asarray(k, dtype=np.float32)
    v = np.asarray(v, dtype=np.float32)

    qT = np.ascontiguousarray(q[0].transpose(1, 2, 0)).astype(np.float16)
    kT = np.ascontiguousarray(k[0].transpose(1, 2, 0)).astype(np.float16)
    # [LK,H,D] -> [H, p, c, d] -> [H, 128, NLC*128]  (p = key % 128)
    vL = np.ascontiguousarray(
        v[0].reshape(NLC, 128, H, D).transpose(2, 1, 0, 3)
    ).reshape(H, 128, NLC * 128).astype(np.float16)

    in_maps = []
    for i in range(N_CORES):
        units = [3 * i + uu for uu in range(UNITS_PER_CORE)]
        heads = [g // 2 for g in units]
        qhs = [g % 2 for g in units]
        in_maps.append({
            "qt": np.ascontiguousarray(
                np.stack([qT[h, :, qh * QSPAN:(qh + 1) * QSPAN]
                          for h, qh in zip(heads, qhs)])),
            "kt": np.ascontiguousarray(np.stack([kT[h] for h in heads])),
            "vl": np.ascontiguousarray(np.stack([vL[h] for h in heads])),
            "bias": bias,
        })
    return in_maps


def _assemble(results):
    out = np.empty((B, LQ, H, D), np.float32)
    for i in range(N_CORES):
        ot = results[i]["ot"].astype(np.float32)              # [3,128,1024]
        nm = results[i]["tr"].astype(np.float32).sum(axis=1)  # [3,1024]
        for uu in range(UNITS_PER_CORE):
            g = 3 * i + uu
            h, qh = g // 2, g % 2
            out[0, qh * QSPAN:(qh + 1) * QSPAN, h, :] = (
                ot[uu] / nm[uu][None, :]).T
    return out


def kernel(q, k, v, frame_seqlen, current_block_start):
    fe = max(0, min(int(frame_seqlen), LK))
    bs = max(0, min(int(current_block_start), LK))
    nc = _get_program(fe, bs)
    in_maps = _host_prep(q, k, v, frame_seqlen, current_block_start)
    res = run_bass_kernel_spmd(nc, in_maps, core_ids=list(range(N_CORES)))
    return _assemble(res.results)


# revision 14
# speedup vs baseline: 2.0920x; 2.0920x over previous
"""Trainium2 Bass kernel for CausalWanSelfAttention (KV-cache-bias attention).

Math: the reference's disjoint-segment attention + LSE merge is exactly
global softmax with a per-key bias b_l (log 0.1 on keys in
[frame_seqlen, current_block_start)).  exp needs no max-subtraction
(scores ~ N(0,1), max ~ 6), so out = (E @ V) / (1^T E) with
E = exp(scale*S + b_l).

Sharding: 24 units = (head h in 0..11, q-half in {0,1}), 3 units per core.
Each unit: 1024 queries x 1 head x all 8192 keys, 64 key chunks of 128.

Device pipeline per unit (matmuls fp16, accumulate fp32 PSUM):
  A:    S^T[l 128, q 1024] = kt-chunk^T @ qt        (N=512 MMs)
  exp:  E = exp(S^T * scale + bias) fp16            (fused ACT instrs)
  B:    O^T[d 128, q 1024] += v-chunk^T @ E         (N=512 MMs)
  R:    in-place fp16 DVE running sum of E slices -> [128,1024] per unit
The kernel is ACT(exp)-bound: Lq*Lk*H/8 = 24.6M exp elements per core at
1 elem/cycle/lane @1.2GHz is ~164us.  Everything else is arranged so ACT
never stalls: exp instructions are fused over 3 half-chunk (512-col)
blocks ([128,1536], 43 instrs/unit instead of 64, amortizing the ~350
cycle ACT instruction overhead), B lags A by a step so the in-order PE
queue always runs the A feeding the next exp first, and the norm never
touches PE/PSUM (R tile is DMA'd out; the host does the final 128-row
cross-partition sum + divide + transpose).

Fusing exp across chunks requires the bias to be constant per ACT
instruction: segment boundaries that are multiples of 128 make the bias
constant per key chunk, so it folds into a [128,1] constant tile and
instructions may span chunks within a segment.  Non-aligned boundaries
fall back to per-chunk (2-block) instructions with a per-partition bias
column DMA'd from the host.  PSUM: s tiles (2x3 banks) + O halves (2).
"""

import math
import sys

for _p in ("/opt/trn_rl_repo",):
    if _p not in sys.path:
        sys.path.insert(0, _p)

import numpy as np

import concourse.bass as bass
import concourse.mybir as mybir
import concourse.tile as tile
from concourse import bacc
from concourse.bass_utils import run_bass_kernel_spmd

F16 = mybir.dt.float16
F32 = mybir.dt.float32

B, LQ, LK, H, D = 1, 2048, 8192, 12, 128
N_CORES = 8
UNITS_PER_CORE = 3          # 24 units = 12 heads x 2 q-halves
QSPAN = 1024                # queries per unit
NLC = LK // 128             # 64 key chunks of 128
SCALE = 1.0 / math.sqrt(D)
LOG_BIAS = math.log(0.1)

_CACHED = None
TIME_LOOP = 1     # timing experiments only: hardware-loop the body N times


def _plan_tiles(fe, bs):
    """Per-unit exp-tile plan: list of (bias, [(chunk, half), ...]).

    Aligned fe/bs: tiles of up to 3 half-chunk blocks, never spanning a
    bias-segment boundary (bias is a compile-time constant per tile).
    Non-aligned: one 2-block tile per chunk with bias=None (use the
    per-partition bias column for that chunk)."""
    if fe % 128 == 0 and bs % 128 == 0:
        tiles = []
        for lo, hi, b in ((0, fe // 128, 0.0),
                          (fe // 128, bs // 128, LOG_BIAS),
                          (bs // 128, NLC, 0.0)):
            blocks = [(c, h) for c in range(lo, hi) for h in range(2)]
            for i in range(0, len(blocks), 3):
                tiles.append((b, blocks[i:i + 3]))
        return tiles, True
    return [(None, [(c, 0), (c, 1)]) for c in range(NLC)], False


def _radds(blocks):
    """Greedy aligned add list for accumulating a tile's E blocks into
    R[128,1024]: returns [(dst_off, src_off, width)]."""
    adds, i = [], 0
    while i < len(blocks):
        c, h = blocks[i]
        if h == 0 and i + 1 < len(blocks) and blocks[i + 1] == (c, 1):
            adds.append((0, i * 512, 1024))
            i += 2
        else:
            adds.append((h * 512, i * 512, 512))
            i += 1
    return adds


def _build_program(fe, bs):
    nc = bacc.Bacc("TRN2", target_bir_lowering=False, debug=False,
                   enable_asserts=False)

    qt_d = nc.dram_tensor("qt", [UNITS_PER_CORE, 128, QSPAN], F16,
                          kind="ExternalInput")
    kt_d = nc.dram_tensor("kt", [UNITS_PER_CORE, 128, LK], F16,
                          kind="ExternalInput")
    # vl host layout: [u, p, c*d] — per-partition contiguous so DMA runs
    # long descriptor lines (p = key index within chunk, c = chunk)
    vl_d = nc.dram_tensor("vl", [UNITS_PER_CORE, 128, NLC * 128], F16,
                          kind="ExternalInput")
    bias_d = nc.dram_tensor("bias", [128, NLC], F32, kind="ExternalInput")
    ot_d = nc.dram_tensor("ot", [UNITS_PER_CORE, 128, QSPAN], F16,
                          kind="ExternalOutput")
    tr_d = nc.dram_tensor("tr", [UNITS_PER_CORE, 128, QSPAN], F16,
                          kind="ExternalOutput")

    qt_ap = qt_d.ap()
    kt_ap = kt_d.ap()
    vl_ap = vl_d.ap().rearrange("u p (c d) -> u p c d", d=128)
    bias_ap = bias_d.ap()
    ot_ap = ot_d.ap()
    tr_ap = tr_d.ap()

    utiles, aligned = _plan_tiles(fe, bs)
    NT = len(utiles)
    swidth = max(len(b) for _, b in utiles) * 512
    # chunk -> {half: (tile, col_off)}
    half_src = {}
    for t, (_b, blocks) in enumerate(utiles):
        for i, (c, h) in enumerate(blocks):
            half_src.setdefault(c, {})[h] = (t, i * 512)

    with tile.TileContext(nc) as tc:
        with (
            tc.tile_pool(name="kt_pool", bufs=2) as kt_pool,
            tc.tile_pool(name="vl_pool", bufs=2) as vl_pool,
            tc.tile_pool(name="qt_pool", bufs=2) as qt_pool,
            tc.tile_pool(name="cn_pool", bufs=1) as cn_pool,
            tc.tile_pool(name="e_pool", bufs=4) as e_pool,
            tc.tile_pool(name="r_pool", bufs=2) as r_pool,
            tc.tile_pool(name="ob_pool", bufs=2) as ob_pool,
            tc.tile_pool(name="s_pool", bufs=2, space="PSUM") as s_pool,
            tc.tile_pool(name="o_pool", bufs=1, space="PSUM") as o_pool,
        ):
            # Warm the ACT exp table before any DMA-dependent work so the
            # ~1.3us table load overlaps the first unit's input DMA.
            warm_t = cn_pool.tile([128, 1], F32, name="warm_t")
            nc.vector.memset(warm_t[:], 0.0)
            nc.scalar.activation(warm_t[:], warm_t[:],
                                 mybir.ActivationFunctionType.Exp)
            bias_c = {}
            if aligned:
                for bv in set(b for b, _ in utiles):
                    bt = cn_pool.tile([128, 1], F32,
                                      name=f"bias_{len(bias_c)}")
                    nc.vector.memset(bt[:], bv)
                    bias_c[bv] = bt
            else:
                bias_t = cn_pool.tile([128, NLC], F32, name="bias_t")

            import contextlib
            loop_cm = (tc.For_i(0, TIME_LOOP, 1) if TIME_LOOP > 1
                       else contextlib.nullcontext())

            loaded = {}

            def load_unit(u):
                # qt first (every tile needs it), then k/v interleaved in
                # eighths so tile 0's compute starts after ~1/8 of the
                # load.  For unit 0 the first chunks ride ahead as a tiny
                # DMA so A(tile 0) isn't gated on a full eighth.
                qt = qt_pool.tile([128, QSPAN], F16, name=f"qt_u{u}", tag="qt")
                nc.sync.dma_start(out=qt[:], in_=qt_ap[u])
                kt = kt_pool.tile([128, LK], F16, name=f"kt_u{u}", tag="kt")
                vl = vl_pool.tile([128, NLC, 128], F16,
                                  name=f"vl_u{u}", tag="vl")
                first = 0
                if u == 0:
                    nc.sync.dma_start(out=kt[:, 0:256], in_=kt_ap[u][:, 0:256])
                    if not aligned:
                        nc.sync.dma_start(out=bias_t[:], in_=bias_ap)
                    first = 256
                for eighth in range(8):
                    slk = slice(max(first, eighth * (LK // 8)),
                                (eighth + 1) * (LK // 8))
                    nc.sync.dma_start(out=kt[:, slk], in_=kt_ap[u][:, slk])
                    slv = bass.ts(eighth, NLC // 8)
                    nc.sync.dma_start(out=vl[:, slv, :], in_=vl_ap[u][:, slv, :])
                loaded[u] = (kt, vl, qt)

            NGT = UNITS_PER_CORE * NT

            with loop_cm:
                load_unit(0)
                # Global software-pipelined stream over exp tiles.  Per
                # step t: A-blocks of tile t, exp(t), then B for chunks
                # fully available in tiles <= t-1 (B lags so PE's in-order
                # queue never delays the A feeding the next exp).
                cur, ot_t, rt = {}, {}, {}
                etiles = {}
                next_b = 0

                def emit_b(ud, dl):
                    kt, vl, qt = cur[ud]
                    if dl == 0:
                        ot_t[ud] = [o_pool.tile([128, 512], F32,
                                                name=f"ot_u{ud}h{h}",
                                                tag=f"ot{h}")
                                    for h in range(2)]
                    if dl == NLC - 1:
                        nc.sync.dma_start(out=tr_ap[ud], in_=rt[ud][:])
                    for h in range(2):
                        et, off = half_src[dl][h]
                        nc.tensor.matmul(
                            ot_t[ud][h][:],
                            lhsT=vl[:, dl, :],
                            rhs=etiles[(ud, et)][:, off:off + 512],
                            start=(dl == 0), stop=(dl == NLC - 1))
                        if dl == NLC - 1:
                            # evacuate/DMA each half as soon as its group
                            # closes so the tail overlaps the other half
                            ot_sb = ob_pool.tile([128, 512], F16,
                                                 name=f"otsb_u{ud}h{h}",
                                                 tag="otsb")
                            nc.vector.tensor_scalar_add(
                                ot_sb[:], ot_t[ud][h][:], 0.0)
                            nc.sync.dma_start(
                                out=ot_ap[ud][:, bass.ts(h, 512)],
                                in_=ot_sb[:])
                    if dl == NLC - 1:
                        ot_t.pop(ud)

                for g in range(NGT + 6):
                    if g < NGT:
                        ug, tg = g // NT, g % NT
                        if tg == 0:
                            cur[ug] = loaded.pop(ug)
                            rt[ug] = r_pool.tile([128, QSPAN], F16,
                                                 name=f"r_u{ug}", tag="r")
                            nc.vector.memset(rt[ug][:], 0.0)
                        kt, vl, qt = cur[ug]
                        b, blocks = utiles[tg]
                        width = len(blocks) * 512
                        s = s_pool.tile([128, swidth], F32)
                        for i, (c, h) in enumerate(blocks):
                            nc.tensor.matmul(
                                s[:, i * 512:(i + 1) * 512],
                                lhsT=kt[:, bass.ts(c, 128)],
                                rhs=qt[:, bass.ts(h, 512)],
                                start=True, stop=True)
                        e = e_pool.tile([128, swidth], F16)
                        bias_arg = (bias_c[b][:] if aligned
                                    else bias_t[:, blocks[0][0]:blocks[0][0] + 1])
                        nc.scalar.activation(
                            e[:, :width], s[:, :width],
                            mybir.ActivationFunctionType.Exp,
                            bias=bias_arg, scale=SCALE)
                        etiles[(ug, tg)] = e
                        r = rt[ug]
                        for dst, src, w in _radds(blocks):
                            nc.vector.tensor_add(
                                r[:, dst:dst + w], r[:, dst:dst + w],
                                e[:, src:src + w])
                        if tg == 6 and ug + 1 < UNITS_PER_CORE:
                            load_unit(ug + 1)  # prefetch next unit
                    # B emission: chunks whose blocks live in tiles
                    # emitted at least one step ago
                    t_done = g - 1
                    while next_b < UNITS_PER_CORE * NLC:
                        ud, dl = next_b // NLC, next_b % NLC
                        lim = max(half_src[dl][0][0], half_src[dl][1][0])
                        if ud * NT + lim <= t_done - 2 or g > NGT + 1:
                            emit_b(ud, dl)
                            next_b += 1
                        else:
                            break
                    # free e tiles fully consumed by B
                    done_c = next_b - 1
                    if done_c >= 0:
                        cut = ((done_c // NLC) * NT +
                               min(half_src[done_c % NLC][0][0],
                                   half_src[done_c % NLC][1][0]))
                        for key in [k for k in etiles
                                    if k[0] * NT + k[1] < cut]:
                            etiles.pop(key)

    nc.compile()
    return nc


def _get_program(fe=1536, bs=6144):
    global _CACHED
    if _CACHED is None:
        _CACHED = {}
    key = (fe, bs, TIME_LOOP)
    if key not in _CACHED:
        _CACHED[key] = _build_program(fe, bs)
    return _CACHED[key]


def _host_prep(q, k, v, frame_seqlen, current_block_start):
    fs = max(0, min(int(frame_seqlen), LK))
    bs = max(0, min(int(current_block_start), LK))
    logw = np.zeros(LK, np.float32)
    logw[fs:bs] = LOG_BIAS
    bias = np.ascontiguousarray(logw.reshape(NLC, 128).T)  # [128, NLC]

    q = np.asarray(q, dtype=np.float32)
    k = np.asarray(k, dtype=np.float32)
    v = np.asarray(v, dtype=np.float32)

    qT = np.ascontiguousarray(q[0].transpose(1, 2, 0)).astype(np.float16)
    kT = np.ascontiguousarray(k[0].transpose(1, 2, 0)).astype(np.float16)
    # [LK,H,D] -> [H, p, c, d] -> [H, 128, NLC*128]  (p = key % 128)
    vL = np.ascontiguousarray(
        v[0].reshape(NLC, 128, H, D).transpose(2, 1, 0, 3)
    ).reshape(H, 128, NLC * 128).astype(np.float16)

    in_maps = []
    for i in range(N_CORES):
        units = [3 * i + uu for uu in range(UNITS_PER_CORE)]
        heads = [g // 2 for g in units]
        qhs = [g % 2 for g in units]
        in_maps.append({
            "qt": np.ascontiguousarray(
                np.stack([qT[h, :, qh * QSPAN:(qh + 1) * QSPAN]
                          for h, qh in zip(heads, qhs)])),
            "kt": np.ascontiguousarray(np.stack([kT[h] for h in heads])),
            "vl": np.ascontiguousarray(np.stack([vL[h] for h in heads])),
            "bias": bias,
        })
    return in_maps


def _assemble(results):
    out = np.empty((B, LQ, H, D), np.float32)
    for i in range(N_CORES):
        ot = results[i]["ot"].astype(np.float32)              # [3,128,1024]
        nm = results[i]["tr"].astype(np.float32).sum(axis=1)  # [3,1024]
        for uu in range(UNITS_PER_CORE):
            g = 3 * i + uu
            h, qh = g // 2, g % 2
            out[0, qh * QSPAN:(qh + 1) * QSPAN, h, :] = (
                ot[uu] / nm[uu][None, :]).T
    return out


def kernel(q, k, v, frame_seqlen, current_block_start):
    fe = max(0, min(int(frame_seqlen), LK))
    bs = max(0, min(int(current_block_start), LK))
    nc = _get_program(fe, bs)
    in_maps = _host_prep(q, k, v, frame_seqlen, current_block_start)
    res = run_bass_kernel_spmd(nc, in_maps, core_ids=list(range(N_CORES)))
    return _assemble(res.results)
